# revision 22
# baseline (speedup 1.0000x reference)
"""Multi-head attention Trainium2 kernel (Bass/Tile, SPMD over 8 cores).

fp16 compute variant: matmul operands in fp16, fp32 PSUM accumulation,
fp32 normalization. Rel err vs fp32 reference ~1e-3.

Sharding: data parallel over batch. Core i computes batches [2i, 2i+2).

Structure per core:
  - Host pre-transposes x -> xT [d, s]; wv -> [d, h*e] (c-major); wq/wk are
    laid out PAIR-major so each head-pair's weights arrive in one DMA and
    pair 0 can start while later weights are still in flight.
  - DMA order: xt0/wv interleaved (feeds batch-0 V projection immediately),
    pair-0 q/k weights, xt1, then remaining pair weights.
  - v projections for BOTH batches run first (covers weight-DMA latency).
  - qT/kT per head-pair: lhsT=W chunk, rhs=xT chunk, accumulate 8 d-chunks.
  - Scores transposed ST[t,s]; exp on ScalarE with bias=-5 (fp16 headroom);
    the two heads of a pair sit at base partitions 0/64 so their K=64
    matmuls row-pack concurrently on the PE.
  - PV with V_aug stationary (ones column -> denominators ride along):
    out^T[e,s] in PSUM; scaled 1/16 copy to fp16 SBUF (alternating
    ScalarE/VectorE to balance engine load); PE-transpose back to [s,e]
    into an 8-slot single-bank PSUM tile; one grouped reciprocal per half
    (4 denominators at once); per-partition multiply (1/16 cancels).
"""

import numpy as np

import concourse.bass as bass
import concourse.mybir as mybir
import concourse.tile as tile
from concourse.bass_utils import run_bass_kernel_spmd
from concourse.masks import make_identity

B, S, D, H, DH = 16, 512, 1024, 16, 64
N_CORES = 8
B_LOC = B // N_CORES  # 2 batches per core
C = D // 128  # 8 contraction chunks over d
TC = S // 128  # 4 chunks over s/t
PAIRS = H // 2
F32 = mybir.dt.float32
FP16 = mybir.dt.float16
SCALE = 1.0 / np.sqrt(np.float32(D))
EXP_BIAS = -5.0  # exp(logit-5): keeps P in fp16 range; cancels in normalize
OSCALE = 1.0 / 16.0  # pre-scale before fp16 transpose; cancels in normalize


def legalize_waits(nc, cap=1):
    """This walrus build supports at most `cap` sync-wait commands per
    instruction; hoist excess waits onto preceding same-engine NoOps."""
    n_split = 0
    for f in nc.m.functions:
        for blk in f.blocks:
            new_insts = []
            for inst in blk.instructions:
                si = getattr(inst, "sync_info", None)
                waits = list(si.on_wait) if si is not None and si.on_wait else []
                if len(waits) > cap:
                    keep, rest = waits[:cap], waits[cap:]
                    while rest:
                        chunk, rest = rest[:cap], rest[cap:]
                        nop = mybir.InstNoOp(
                            name=f"I-waitsplit-{nc.next_id()}", ins=[], outs=[]
                        )
                        nop.engine = inst.engine
                        nop.sync_info = mybir.SyncInfo(on_wait=chunk, on_update=[])
                        nc.register_instruction(nop, overwrite=True)
                        new_insts.append(nop)
                        n_split += 1
                    si.on_wait = keep
                new_insts.append(inst)
            blk.instructions[:] = new_insts
    return n_split


def build_program():
    nc = bass.Bass()
    xt_d = nc.declare_dram_parameter("xt", [B_LOC, C, 128, S], FP16, isOutput=False)
    wq_d = nc.declare_dram_parameter("wq", [PAIRS, 128, C * 128], FP16, isOutput=False)
    wk_d = nc.declare_dram_parameter("wk", [PAIRS, 128, C * 128], FP16, isOutput=False)
    wv_d = nc.declare_dram_parameter("wv", [C, 128, D], FP16, isOutput=False)
    out_d = nc.declare_dram_parameter("out", [B_LOC, S, D], F32, isOutput=True)

    with tile.TileContext(nc) as tc:
        with (
            tc.tile_pool(name="sb", bufs=1) as sb,
            tc.tile_pool(name="ps", bufs=1, space="PSUM") as ps_pool,
        ):
            wpool = xpool = vpool = qkpool = ppool = opool = rpool = ovpool = sb
            psmm = stp = psout = pstrp = ps_pool
            ident = ovpool.tile([128, 128], FP16, tag="ident", bufs=1)
            make_identity(nc, ident)
            exp_bias = ovpool.tile([128, 1], F32, tag="expbias", bufs=1)
            nc.vector.memset(exp_bias, EXP_BIAS)
            # 8-slot transpose landing zone: one PSUM bank, subtile deps
            pstr_t = pstrp.tile([128, 8, 128], FP16, tag="ot", bufs=1)
            # PE warm-up: dense dependency-free transposes so HAM ramps the
            # tensor engine toward full rate while the first input DMAs land
            for i in range(16):
                nc.tensor.transpose(pstr_t[:, i % 8, :], ident, ident)

            # ---- inputs ----
            xts = [
                xpool.tile([128, C, S], FP16, tag=f"xt{b}", name=f"xt{b}")
                for b in range(B_LOC)
            ]
            wq_sb = wpool.tile([128, PAIRS, C * 128], FP16, tag="wq")
            wk_sb = wpool.tile([128, PAIRS, C * 128], FP16, tag="wk")
            wv_sb = wpool.tile([128, C, D], FP16, tag="wv")
            # xt0 + wv interleaved: batch-0 V projection starts ~immediately
            for c in range(C):
                nc.sync.dma_start(out=xts[0][:, c, :], in_=xt_d[0, c])
                nc.sync.dma_start(out=wv_sb[:, c, :], in_=wv_d[c])
            # pair-0 q/k weights early so attention can start right after
            nc.sync.dma_start(out=wq_sb[:, 0], in_=wq_d[0])
            nc.sync.dma_start(out=wk_sb[:, 0], in_=wk_d[0])
            # xt1 before remaining weights: batch-1 V projection needs it
            for c in range(C):
                nc.sync.dma_start(out=xts[1][:, c, :], in_=xt_d[1, c])
            for p in range(1, PAIRS):
                nc.sync.dma_start(out=wq_sb[:, p], in_=wq_d[p])
                nc.sync.dma_start(out=wk_sb[:, p], in_=wk_d[p])

            # ---- v projections for BOTH batches up front ----
            # V_aug layout [128(t), h, 64(e) + ones + pad]
            vaugs = {}
            for b in range(B_LOC):
                vaugs[b] = [
                    vpool.tile(
                        [128, H, DH + 2], FP16, tag=f"vaug{b}", name=f"vaug{b}_{t}", bufs=4
                    )
                    for t in range(TC)
                ]
                for t in range(TC):
                    nc.vector.memset(vaugs[b][t][:, :, DH : DH + 2], 1.0)
            for b in range(B_LOC):
                for t in range(TC):
                    # both halves in one 2-bank tile; halves interleaved per
                    # chunk so the PE consumes input DMAs as they land
                    ps = stp.tile([128, 2, 512], F32, tag="st", bufs=2)
                    for c in range(C):
                        for half in range(2):
                            nc.tensor.matmul(
                                ps[:, half, :],
                                lhsT=xts[b][:, c, t * 128 : (t + 1) * 128],
                                rhs=wv_sb[:, c, half * 512 : (half + 1) * 512],
                                start=(c == 0),
                                stop=(c == C - 1),
                            )
                    if t % 2 == 0:
                        nc.scalar.copy(
                            vaugs[b][t][:, :, 0:DH],
                            ps.rearrange("p a (h e) -> p (a h) e", h=8),
                        )
                    else:
                        nc.vector.tensor_copy(
                            vaugs[b][t][:, :, 0:DH],
                            ps.rearrange("p a (h e) -> p (a h) e", h=8),
                        )

            # ---- per-batch, per-head-pair attention ----
            # Stage-split software pipeline: emit stage1 (qk projections,
            # scores, exp) for unit i, stage2 (PV, normalize, DMA out) for
            # unit i-1 -- so the tail still has PV work to overlap the last
            # exps instead of draining serially.
            osbs = {
                b: [
                    opool.tile(
                        [128, D], F32, tag=f"osb{b}", name=f"osb{b}_{sc}", bufs=4
                    )
                    for sc in range(TC)
                ]
                for b in range(B_LOC)
            }
            units = [(b, pair) for b in range(B_LOC) for pair in range(PAIRS)]
            state = {}
            slot_ctr = 0  # cycles the 8 pstr transpose slots (4 per half)

            def stage1(b, pair):
                xt_sb = xts[b]
                qt = qkpool.tile([128, S], FP16, tag="qt", bufs=4, name="qt")
                kt = qkpool.tile([128, S], FP16, tag="kt", bufs=4, name="kt")
                for w_sb, dst in ((wq_sb, qt), (wk_sb, kt)):
                    ps = psmm.tile([128, 512], F32, tag="mm", bufs=2, name="ps")
                    for c in range(C):
                        nc.tensor.matmul(
                            ps,
                            lhsT=w_sb[:, pair, c * 128 : (c + 1) * 128],
                            rhs=xt_sb[:, c, :],
                            start=(c == 0),
                            stop=(c == C - 1),
                        )
                    nc.vector.tensor_copy(dst, ps)
                p_tiles = {}
                for t in range(TC):
                    ps = stp.tile([128, 2, 512], F32, tag="st", bufs=2, name="ps")
                    for half in range(2):
                        lo, hi = 64 * half, 64 * (half + 1)
                        nc.tensor.matmul(
                            ps[:, half, :],
                            lhsT=kt[lo:hi, t * 128 : (t + 1) * 128],
                            rhs=qt[lo:hi, :],
                            start=True,
                            stop=True,
                        )
                    pt = ppool.tile([128, 2, 512], FP16, tag="p", bufs=10, name="pt")
                    nc.scalar.activation(
                        pt.rearrange("p a b -> p (a b)"),
                        ps.rearrange("p a b -> p (a b)"),
                        mybir.ActivationFunctionType.Exp,
                        scale=float(SCALE),
                        bias=exp_bias[:, :],
                    )
                    for half in range(2):
                        p_tiles[(half, t)] = pt[:, half, :]
                state[(b, pair)] = p_tiles

            def stage2(b, pair, last_pair, near_tail=False):
                nonlocal slot_ctr
                vaug = vaugs[b]
                osb = osbs[b]
                p_tiles = state.pop((b, pair))
                for half in range(2):
                    h = pair * 2 + half
                    oaug = psout.tile([DH + 2, 512], F32, tag="o", bufs=1, name="oaug")
                    for t in range(TC):
                        nc.tensor.matmul(
                            oaug,
                            lhsT=vaug[t][:, h, :],
                            rhs=p_tiles[(half, t)],
                            start=(t == 0),
                            stop=(t == TC - 1),
                        )
                    # scaled fp16 copy (PSUM->SBUF); alternate engine to
                    # balance ScalarE/VectorE load; 1/16 cancels later
                    oaug_sb = ovpool.tile(
                        [DH + 2, 512], FP16, tag="oaug", bufs=3, name="oaug_sb"
                    )
                    if half == 0:
                        nc.scalar.mul(oaug_sb, oaug, OSCALE)
                    else:
                        nc.vector.tensor_scalar_mul(oaug_sb, oaug, OSCALE)
                    g = (slot_ctr % 2) * 4
                    slot_ctr += 1
                    recip = rpool.tile([128, TC, 1], F32, tag="r", bufs=8, name="recip")
                    if last_pair:
                        for sc in range(TC):
                            nc.tensor.transpose(
                                pstr_t[:, g + sc, 0 : DH + 2],
                                oaug_sb[:, sc * 128 : (sc + 1) * 128],
                                ident[: DH + 2, : DH + 2],
                            )
                            nc.vector.reciprocal(
                                recip[:, sc, :], pstr_t[:, g + sc, DH : DH + 1]
                            )
                    else:
                        for sc in range(TC):
                            nc.tensor.transpose(
                                pstr_t[:, g + sc, 0 : DH + 2],
                                oaug_sb[:, sc * 128 : (sc + 1) * 128],
                                ident[: DH + 2, : DH + 2],
                            )
                        # one grouped reciprocal covers all 4 denominators
                        nc.vector.reciprocal(
                            recip, pstr_t[:, g : g + TC, DH : DH + 1]
                        )
                    for sc in range(TC):
                        if last_pair and sc % 2 == 0:
                            nc.scalar.activation(
                                osb[sc][:, h * DH : (h + 1) * DH],
                                pstr_t[:, g + sc, 0:DH],
                                mybir.ActivationFunctionType.Copy,
                                scale=recip[:, sc, :],
                            )
                        else:
                            nc.vector.tensor_scalar_mul(
                                osb[sc][:, h * DH : (h + 1) * DH],
                                pstr_t[:, g + sc, 0:DH],
                                recip[:, sc, :],
                            )
                # last pair: fan its 4 DMAs across idle queues so the
                # final drain isn't serialized behind one queue
                if last_pair:
                    dma_engines = (nc.sync, nc.gpsimd, nc.scalar, nc.sync)
                elif near_tail:
                    dma_engines = (nc.sync, nc.gpsimd, nc.sync, nc.gpsimd)
                else:
                    dma_engines = (nc.sync,) * 4
                for sc in range(TC):
                    dma_engines[sc].dma_start(
                        out=out_d[
                            b, sc * 128 : (sc + 1) * 128, pair * 128 : (pair + 1) * 128
                        ],
                        in_=osb[sc][:, pair * 128 : (pair + 1) * 128],
                    )

            n_u = len(units)
            for i, (b, pair) in enumerate(units):
                stage1(b, pair)
                if i >= 1:
                    stage2(
                        *units[i - 1], last_pair=False, near_tail=(i == n_u - 1)
                    )
            stage2(*units[-1], last_pair=True)

    legalize_waits(nc)
    return nc


def _prep_inputs(x, Wq, Wk, Wv):
    x = np.ascontiguousarray(np.asarray(x, dtype=np.float32))
    # x [B, S, D] -> per-core xT [B_LOC, C, 128, S]
    xt = x.reshape(N_CORES, B_LOC, S, D).transpose(0, 1, 3, 2)
    xt = np.ascontiguousarray(xt).reshape(N_CORES, B_LOC, C, 128, S).astype(np.float16)

    def dmaj(W):
        # [H, D, DH] -> [D, H*DH] (d-major)
        return np.ascontiguousarray(np.asarray(W, dtype=np.float32).transpose(1, 0, 2)).reshape(D, H * DH)

    # wq/wk: pair-major [PAIRS, 128(d in chunk), C*128] so one DMA per pair
    def pair_major(W):
        Wt = dmaj(W)  # [d, o]
        return (
            Wt.reshape(C, 128, PAIRS, 128)
            .transpose(2, 1, 0, 3)
            .reshape(PAIRS, 128, C * 128)
            .astype(np.float16)
        )

    wq_p = np.ascontiguousarray(pair_major(Wq))
    wk_p = np.ascontiguousarray(pair_major(Wk))
    wv_p = np.ascontiguousarray(dmaj(Wv).reshape(C, 128, H * DH).astype(np.float16))
    return xt, wq_p, wk_p, wv_p


_PROGRAM = None


def _get_program():
    global _PROGRAM
    if _PROGRAM is None:
        _PROGRAM = build_program()
    return _PROGRAM


def run(x, Wq, Wk, Wv, trace=False, nc=None):
    xt, wq_p, wk_p, wv_p = _prep_inputs(x, Wq, Wk, Wv)
    if nc is None:
        nc = _get_program()
    in_maps = [
        {"xt": xt[i], "wq": wq_p, "wk": wk_p, "wv": wv_p} for i in range(N_CORES)
    ]
    res = run_bass_kernel_spmd(nc, in_maps, list(range(N_CORES)), trace=trace)
    out = np.concatenate([res.results[i]["out"] for i in range(N_CORES)], axis=0)
    return out, res


def kernel(x, Wq, Wk, Wv):
    out, _ = run(x, Wq, Wk, Wv, trace=False)
    return out


# revision 23
# speedup vs baseline: 1.0256x; 1.0256x over previous
"""Multi-head attention Trainium2 kernel (Bass/Tile, SPMD over 8 cores).

fp16 compute variant: matmul operands in fp16, fp32 PSUM accumulation,
fp32 normalization. Rel err vs fp32 reference ~1e-3.

Sharding: data parallel over batch. Core i computes batches [2i, 2i+2).

Structure per core:
  - Host pre-transposes x -> xT [d, s]; wv -> [d, h*e] (c-major); wq/wk are
    laid out PAIR-major so each head-pair's weights arrive in one DMA and
    pair 0 can start while later weights are still in flight.
  - DMA order: xt0/wv interleaved (feeds batch-0 V projection immediately),
    pair-0 q/k weights, xt1, then remaining pair weights.
  - v projections for BOTH batches run first (covers weight-DMA latency).
  - qT/kT per head-pair: lhsT=W chunk, rhs=xT chunk, accumulate 8 d-chunks.
  - Scores transposed ST[t,s]; exp on ScalarE with bias=-5 (fp16 headroom);
    the two heads of a pair sit at base partitions 0/64 so their K=64
    matmuls row-pack concurrently on the PE.
  - PV with V_aug stationary (ones column -> denominators ride along):
    out^T[e,s] in PSUM; scaled 1/16 copy to fp16 SBUF (alternating
    ScalarE/VectorE to balance engine load); PE-transpose back to [s,e]
    into an 8-slot single-bank PSUM tile; one grouped reciprocal per half
    (4 denominators at once); per-partition multiply (1/16 cancels).
"""

import numpy as np

import concourse.bass as bass
import concourse.mybir as mybir
import concourse.tile as tile
from concourse.bass_utils import run_bass_kernel_spmd
from concourse.masks import make_identity

B, S, D, H, DH = 16, 512, 1024, 16, 64
N_CORES = 8
B_LOC = B // N_CORES  # 2 batches per core
C = D // 128  # 8 contraction chunks over d
TC = S // 128  # 4 chunks over s/t
PAIRS = H // 2
F32 = mybir.dt.float32
FP16 = mybir.dt.float16
SCALE = 1.0 / np.sqrt(np.float32(D))
EXP_BIAS = -5.0  # exp(logit-5): keeps P in fp16 range; cancels in normalize
OSCALE = 1.0 / 16.0  # pre-scale before fp16 transpose; cancels in normalize


def legalize_waits(nc, cap=1):
    """This walrus build supports at most `cap` sync-wait commands per
    instruction; hoist excess waits onto preceding same-engine NoOps."""
    n_split = 0
    for f in nc.m.functions:
        for blk in f.blocks:
            new_insts = []
            for inst in blk.instructions:
                si = getattr(inst, "sync_info", None)
                waits = list(si.on_wait) if si is not None and si.on_wait else []
                if len(waits) > cap:
                    keep, rest = waits[:cap], waits[cap:]
                    while rest:
                        chunk, rest = rest[:cap], rest[cap:]
                        nop = mybir.InstNoOp(
                            name=f"I-waitsplit-{nc.next_id()}", ins=[], outs=[]
                        )
                        nop.engine = inst.engine
                        nop.sync_info = mybir.SyncInfo(on_wait=chunk, on_update=[])
                        nc.register_instruction(nop, overwrite=True)
                        new_insts.append(nop)
                        n_split += 1
                    si.on_wait = keep
                new_insts.append(inst)
            blk.instructions[:] = new_insts
    return n_split


def build_program():
    nc = bass.Bass()
    xt_d = nc.declare_dram_parameter("xt", [B_LOC, C, 128, S], FP16, isOutput=False)
    wq_d = nc.declare_dram_parameter("wq", [PAIRS, 128, C * 128], FP16, isOutput=False)
    wk_d = nc.declare_dram_parameter("wk", [PAIRS, 128, C * 128], FP16, isOutput=False)
    wv_d = nc.declare_dram_parameter("wv", [C, 128, D], FP16, isOutput=False)
    out_d = nc.declare_dram_parameter("out", [B_LOC, S, D], F32, isOutput=True)

    with tile.TileContext(nc) as tc:
        with (
            tc.tile_pool(name="sb", bufs=1) as sb,
            tc.tile_pool(name="ps", bufs=1, space="PSUM") as ps_pool,
        ):
            wpool = xpool = vpool = qkpool = ppool = opool = rpool = ovpool = sb
            psmm = stp = psout = pstrp = ps_pool
            ident = ovpool.tile([128, 128], FP16, tag="ident", bufs=1)
            make_identity(nc, ident)
            exp_bias = ovpool.tile([128, 1], F32, tag="expbias", bufs=1)
            nc.vector.memset(exp_bias, EXP_BIAS)
            # 8-slot transpose landing zone: one PSUM bank, subtile deps
            pstr_t = pstrp.tile([128, 8, 128], FP16, tag="ot", bufs=1)
            # PE warm-up: dense dependency-free transposes so HAM ramps the
            # tensor engine toward full rate while the first input DMAs land
            for i in range(16):
                nc.tensor.transpose(pstr_t[:, i % 8, :], ident, ident)

            # ---- inputs ----
            xts = [
                xpool.tile([128, C, S], FP16, tag=f"xt{b}", name=f"xt{b}")
                for b in range(B_LOC)
            ]
            wq_sb = wpool.tile([128, PAIRS, C * 128], FP16, tag="wq")
            wk_sb = wpool.tile([128, PAIRS, C * 128], FP16, tag="wk")
            wv_sb = wpool.tile([128, C, D], FP16, tag="wv")
            # xt0 + wv interleaved: batch-0 V projection starts ~immediately
            for c in range(C):
                nc.sync.dma_start(out=xts[0][:, c, :], in_=xt_d[0, c])
                nc.sync.dma_start(out=wv_sb[:, c, :], in_=wv_d[c])
            # pair-0 q/k weights early so attention can start right after
            nc.sync.dma_start(out=wq_sb[:, 0], in_=wq_d[0])
            nc.sync.dma_start(out=wk_sb[:, 0], in_=wk_d[0])
            # xt1 before remaining weights: batch-1 V projection needs it
            for c in range(C):
                nc.sync.dma_start(out=xts[1][:, c, :], in_=xt_d[1, c])
            for p in range(1, PAIRS):
                nc.sync.dma_start(out=wq_sb[:, p], in_=wq_d[p])
                nc.sync.dma_start(out=wk_sb[:, p], in_=wk_d[p])

            # ---- v projections for BOTH batches up front ----
            # V_aug layout [128(t), h, 64(e) + ones + pad]
            vaugs = {}
            for b in range(B_LOC):
                vaugs[b] = [
                    vpool.tile(
                        [128, H, DH + 2], FP16, tag=f"vaug{b}", name=f"vaug{b}_{t}", bufs=4
                    )
                    for t in range(TC)
                ]
                for t in range(TC):
                    nc.vector.memset(vaugs[b][t][:, :, DH : DH + 2], 1.0)
            for b in range(B_LOC):
                for t in range(TC):
                    # both halves in one 2-bank tile; halves interleaved per
                    # chunk so the PE consumes input DMAs as they land
                    ps = stp.tile([128, 2, 512], F32, tag="st", bufs=2)
                    for c in range(C):
                        for half in range(2):
                            nc.tensor.matmul(
                                ps[:, half, :],
                                lhsT=xts[b][:, c, t * 128 : (t + 1) * 128],
                                rhs=wv_sb[:, c, half * 512 : (half + 1) * 512],
                                start=(c == 0),
                                stop=(c == C - 1),
                            )
                    if t % 2 == 0:
                        nc.scalar.copy(
                            vaugs[b][t][:, :, 0:DH],
                            ps.rearrange("p a (h e) -> p (a h) e", h=8),
                        )
                    else:
                        nc.vector.tensor_copy(
                            vaugs[b][t][:, :, 0:DH],
                            ps.rearrange("p a (h e) -> p (a h) e", h=8),
                        )

            # ---- per-batch, per-head-pair attention ----
            # Stage-split software pipeline: emit stage1 (qk projections,
            # scores, exp) for unit i, stage2 (PV, normalize, DMA out) for
            # unit i-1 -- so the tail still has PV work to overlap the last
            # exps instead of draining serially.
            osbs = {
                b: [
                    opool.tile(
                        [128, D], F32, tag=f"osb{b}", name=f"osb{b}_{sc}", bufs=4
                    )
                    for sc in range(TC)
                ]
                for b in range(B_LOC)
            }
            units = [(b, pair) for b in range(B_LOC) for pair in range(PAIRS)]
            state = {}
            slot_ctr = 0  # cycles the 8 pstr transpose slots (4 per half)

            def stage1(b, pair):
                xt_sb = xts[b]
                qt = qkpool.tile([128, S], FP16, tag="qt", bufs=4, name="qt")
                kt = qkpool.tile([128, S], FP16, tag="kt", bufs=4, name="kt")
                for w_sb, dst in ((wq_sb, qt), (wk_sb, kt)):
                    ps = psmm.tile([128, 512], F32, tag="mm", bufs=2, name="ps")
                    for c in range(C):
                        nc.tensor.matmul(
                            ps,
                            lhsT=w_sb[:, pair, c * 128 : (c + 1) * 128],
                            rhs=xt_sb[:, c, :],
                            start=(c == 0),
                            stop=(c == C - 1),
                        )
                    nc.vector.tensor_copy(dst, ps)
                p_tiles = {}
                for t in range(TC):
                    ps = stp.tile([128, 2, 512], F32, tag="st", bufs=2, name="ps")
                    for half in range(2):
                        lo, hi = 64 * half, 64 * (half + 1)
                        nc.tensor.matmul(
                            ps[:, half, :],
                            lhsT=kt[lo:hi, t * 128 : (t + 1) * 128],
                            rhs=qt[lo:hi, :],
                            start=True,
                            stop=True,
                        )
                    pt = ppool.tile([128, 2, 512], FP16, tag="p", bufs=10, name="pt")
                    nc.scalar.activation(
                        pt.rearrange("p a b -> p (a b)"),
                        ps.rearrange("p a b -> p (a b)"),
                        mybir.ActivationFunctionType.Exp,
                        scale=float(SCALE),
                        bias=exp_bias[:, :],
                    )
                    for half in range(2):
                        p_tiles[(half, t)] = pt[:, half, :]
                state[(b, pair)] = p_tiles

            def stage2(b, pair, last_pair):
                nonlocal slot_ctr
                vaug = vaugs[b]
                osb = osbs[b]
                p_tiles = state.pop((b, pair))
                for half in range(2):
                    h = pair * 2 + half
                    oaug = psout.tile([DH + 2, 512], F32, tag="o", bufs=1, name="oaug")
                    for t in range(TC):
                        nc.tensor.matmul(
                            oaug,
                            lhsT=vaug[t][:, h, :],
                            rhs=p_tiles[(half, t)],
                            start=(t == 0),
                            stop=(t == TC - 1),
                        )
                    # scaled fp16 copy (PSUM->SBUF); alternate engine to
                    # balance ScalarE/VectorE load; 1/16 cancels later
                    oaug_sb = ovpool.tile(
                        [DH + 2, 512], FP16, tag="oaug", bufs=3, name="oaug_sb"
                    )
                    if half == 0 or last_pair:
                        nc.scalar.mul(oaug_sb, oaug, OSCALE)
                    else:
                        nc.vector.tensor_scalar_mul(oaug_sb, oaug, OSCALE)
                    g = (slot_ctr % 2) * 4
                    slot_ctr += 1
                    for sc in range(TC):
                        nc.tensor.transpose(
                            pstr_t[:, g + sc, 0 : DH + 2],
                            oaug_sb[:, sc * 128 : (sc + 1) * 128],
                            ident[: DH + 2, : DH + 2],
                        )
                    # one grouped reciprocal covers all 4 denominators
                    recip = rpool.tile([128, TC, 1], F32, tag="r", bufs=8, name="recip")
                    nc.vector.reciprocal(recip, pstr_t[:, g : g + TC, DH : DH + 1])
                    for sc in range(TC):
                        if last_pair and sc % 2 == 0:
                            nc.scalar.activation(
                                osb[sc][:, h * DH : (h + 1) * DH],
                                pstr_t[:, g + sc, 0:DH],
                                mybir.ActivationFunctionType.Copy,
                                scale=recip[:, sc, :],
                            )
                        else:
                            nc.vector.tensor_scalar_mul(
                                osb[sc][:, h * DH : (h + 1) * DH],
                                pstr_t[:, g + sc, 0:DH],
                                recip[:, sc, :],
                            )
                # last pair: fan its 4 DMAs across idle queues so the
                # final drain isn't serialized behind one queue
                dma_engines = (
                    (nc.sync, nc.gpsimd, nc.scalar, nc.sync)
                    if last_pair
                    else (nc.sync,) * 4
                )
                for sc in range(TC):
                    dma_engines[sc].dma_start(
                        out=out_d[
                            b, sc * 128 : (sc + 1) * 128, pair * 128 : (pair + 1) * 128
                        ],
                        in_=osb[sc][:, pair * 128 : (pair + 1) * 128],
                    )

            for i, (b, pair) in enumerate(units):
                stage1(b, pair)
                if i >= 1:
                    stage2(*units[i - 1], last_pair=False)
            stage2(*units[-1], last_pair=True)

    legalize_waits(nc)
    return nc


def _prep_inputs(x, Wq, Wk, Wv):
    x = np.ascontiguousarray(np.asarray(x, dtype=np.float32))
    # x [B, S, D] -> per-core xT [B_LOC, C, 128, S]
    xt = x.reshape(N_CORES, B_LOC, S, D).transpose(0, 1, 3, 2)
    xt = np.ascontiguousarray(xt).reshape(N_CORES, B_LOC, C, 128, S).astype(np.float16)

    def dmaj(W):
        # [H, D, DH] -> [D, H*DH] (d-major)
        return np.ascontiguousarray(np.asarray(W, dtype=np.float32).transpose(1, 0, 2)).reshape(D, H * DH)

    # wq/wk: pair-major [PAIRS, 128(d in chunk), C*128] so one DMA per pair
    def pair_major(W):
        Wt = dmaj(W)  # [d, o]
        return (
            Wt.reshape(C, 128, PAIRS, 128)
            .transpose(2, 1, 0, 3)
            .reshape(PAIRS, 128, C * 128)
            .astype(np.float16)
        )

    wq_p = np.ascontiguousarray(pair_major(Wq))
    wk_p = np.ascontiguousarray(pair_major(Wk))
    wv_p = np.ascontiguousarray(dmaj(Wv).reshape(C, 128, H * DH).astype(np.float16))
    return xt, wq_p, wk_p, wv_p


_PROGRAM = None


def _get_program():
    global _PROGRAM
    if _PROGRAM is None:
        _PROGRAM = build_program()
    return _PROGRAM


def run(x, Wq, Wk, Wv, trace=False, nc=None):
    xt, wq_p, wk_p, wv_p = _prep_inputs(x, Wq, Wk, Wv)
    if nc is None:
        nc = _get_program()
    in_maps = [
        {"xt": xt[i], "wq": wq_p, "wk": wk_p, "wv": wv_p} for i in range(N_CORES)
    ]
    res = run_bass_kernel_spmd(nc, in_maps, list(range(N_CORES)), trace=trace)
    out = np.concatenate([res.results[i]["out"] for i in range(N_CORES)], axis=0)
    return out, res


def kernel(x, Wq, Wk, Wv):
    out, _ = run(x, Wq, Wk, Wv, trace=False)
    return out
